# revision 71
# baseline (speedup 1.0000x reference)
"""Trainium2 Bass kernel for nn_DepthwiseXCorr (SiamRPN-style depthwise-xcorr head).

Pipeline per sample (B=128 sharded 16/core across 8 cores, pure data parallel):
  k = relu(bn1(conv3x3(kernel_in, w_ck)))      [256, 5, 5]
  s = relu(bn2(conv3x3(search_in, w_cs)))      [256, 29, 29]
  feat = depthwise_xcorr(s, k)                 [256, 25, 25]
  h = relu(bn3(conv1x1(feat, w_h1)))           [256, 25, 25]
  out = conv1x1(h, w_h2) + b_h2                [10, 25, 25]

Implementation notes:
  - BN scale folded into conv weights host-side; BN shift + ReLU applied by
    the ACT engine on the PSUM->SBUF eviction (activation = relu(x*1 + bias)).
  - Convs are implicit GEMM on TensorE.  conv_search runs in bf16 (1 cyc/row,
    same PE rate as fp32r, but exempt from the fp32r even-innermost-run ISA
    rule, so the PSUM tiles are 29 wide instead of 30) with fp32 PSUM
    accumulation; conv_kernel inputs (kin im2col + wk) are bf16 to halve
    their startup DMA time.
  - Depthwise xcorr (32 (sample,chunk) units/core) is split across the two
    engines that can do per-partition-scalar MACs at full rate: 14 units run
    on the PE as diagonal-weight matmuls (25 matmuls/unit, diag tiles built
    by Pool broadcast tensor_tensor one sample ahead); 18 units run as
    25-tap scalar_tensor_tensor chains on DVE (bf16 windows, fp32
    accumulator).  GPSIMD supports no scalar_tensor_tensor on real HW, and
    ACT-product + gpsimd DMA-accumulate lanes lose to SWDGE descriptor-gen
    cost, so both stay out of the steady state (M_UNITS kept for reference).
  - The head (h1/h2) is software-pipelined 3 samples behind conv/xcorr so
    DVE's unit lag never stalls the PE (the PE p-state drops on any stall);
    the tail drains two heads before the last sample's xcorr so their PSUM
    ping-pong overlaps xcorr matmuls.
  - DMA discipline: transfers serialize on the DMA engines, so startup loads
    are ordered strictly by first use; search tiles are host-padded to
    [31,32] so each load is one descriptor per partition; y stores issue on
    the ACT ring right after their eviction so they never wait.
"""

import numpy as np

EPS = 1e-5
N_CORES = 8
B = 128
B_PER = B // N_CORES  # 16
CIN = 256
H = 256
COUT = 10

_NC_CACHE = {}

# unit u = sample*2 + chunk.  Three xcorr lanes:
#   P: PE diagonal-weight matmuls (diag tiles built by Pool broadcasts)
#   M: ACT per-partition-scaled products + gpsimd DMA-accumulate chains
#   V: DVE 25-tap scalar_tensor_tensor chains
# GPSIMD itself supports no scalar_tensor_tensor on real HW.
PE_UNITS = (5, 7, 9, 11, 13, 15, 17, 19, 21, 23, 25, 27, 29, 30, 31)
M_UNITS = ()


def _build_nc(b_per=B_PER):
    """Build the Bass program for one core processing `b_per` samples."""
    import concourse.bacc as bacc
    import concourse.mybir as mybir
    import concourse.tile as tile

    dt = mybir.dt
    f32 = dt.float32
    f32r = dt.float32r
    AF = mybir.ActivationFunctionType
    ALU = mybir.AluOpType

    nc = bacc.Bacc("TRN2", target_bir_lowering=False, debug=False)

    # ---- DRAM tensors (shapes match SBUF tiles exactly; host pre-transposes) ----
    search_d = nc.dram_tensor("search", [b_per, 2, 128, 31, 32], dt.bfloat16, kind="ExternalInput")
    kin_d = nc.dram_tensor("kin", [2, 128, 9, b_per, 25], dt.bfloat16, kind="ExternalInput")
    wk_d = nc.dram_tensor("wk", [2, 128, 18, 128], dt.bfloat16, kind="ExternalInput")
    ws_d = nc.dram_tensor("ws", [2, 128, 18, 128], dt.bfloat16, kind="ExternalInput")
    w1_d = nc.dram_tensor("w1", [2, 128, 2, 128], f32r, kind="ExternalInput")
    w2_d = nc.dram_tensor("w2", [2, 128, 10], dt.bfloat16, kind="ExternalInput")
    eye_d = nc.dram_tensor("eye", [128, 128], dt.bfloat16, kind="ExternalInput")
    b1_d = nc.dram_tensor("b1s", [128, 2], f32, kind="ExternalInput")
    b2_d = nc.dram_tensor("b2s", [128, 2], f32, kind="ExternalInput")
    b3_d = nc.dram_tensor("b3s", [128, 2], f32, kind="ExternalInput")
    bh_d = nc.dram_tensor("bhs", [10, 1], f32, kind="ExternalInput")
    y_d = nc.dram_tensor("y", [b_per, 10, 25, 25], f32, kind="ExternalOutput")

    TAPS3 = [(dy, dx) for dy in range(3) for dx in range(3)]
    TAPS5 = [(dy, dx) for dy in range(5) for dx in range(5)]
    # conv_search output row tiling: 29 rows -> two PSUM tiles (N = 435 / 406)
    CS_ROWS = [(0, 15), (15, 14)]
    # h1/h2/PE-xcorr output row tiling: 25 rows -> two PSUM tiles (N = 325 / 300)
    H_ROWS = [(0, 13), (13, 12)]

    with tile.TileContext(nc) as tc:
        with (
            tc.tile_pool(name="wpool", bufs=1) as wpool,
            tc.tile_pool(name="kpool", bufs=1) as kpool,
            tc.tile_pool(name="spool", bufs=8) as spool,
            tc.tile_pool(name="fpool", bufs=12) as fpool,
            tc.tile_pool(name="hpool", bufs=5) as hpool,
            tc.tile_pool(name="sfpool", bufs=8) as sfpool,
            tc.tile_pool(name="opool", bufs=3) as opool,
            tc.tile_pool(name="dpool", bufs=3) as dpool,
            tc.tile_pool(name="ppool", bufs=5) as ppool,
            tc.tile_pool(name="ps_cs", bufs=2, space="PSUM") as ps_cs,
            tc.tile_pool(name="ps_h", bufs=4, space="PSUM") as ps_h,
            tc.tile_pool(name="ps_h2", bufs=2, space="PSUM") as ps_h2,
        ):
            sin_tiles = {}

            def emit_load(b):
                sin = []
                for ci in range(2):
                    st = spool.tile([128, 31, 32], dt.bfloat16, tag="sin")
                    nc.sync.dma_start(st[:], search_d[b, ci])
                    sin.append(st)
                sin_tiles[b] = sin

            # ---- startup DMA schedule (ACT ring: kin + biases; Pool ring:
            # ws + wk + eye; SP ring: search samples + head weights) ----
            kin_sb = []
            wk_sb = []
            ws_sb = []
            w1_sb = []
            w2_sb = []
            # transfers serialize on the DMA engines, so order strictly by
            # first use: conv_search[0] inputs, then phase-K inputs, then the
            # second sample and head weights
            b2_sb = wpool.tile([128, 2], f32, tag="b2")
            nc.scalar.dma_start(b2_sb[:], b2_d[:])
            # interleave so conv_search[0]'s first chunk (ws0 + sin0c0) heads
            # the serialized DMA queue
            sin0 = []
            wst = wpool.tile([128, 18, 128], dt.bfloat16, tag="ws0")
            nc.gpsimd.dma_start(wst[:], ws_d[0])
            ws_sb.append(wst)
            st = spool.tile([128, 31, 32], dt.bfloat16, tag="sin")
            nc.sync.dma_start(st[:], search_d[0, 0])
            sin0.append(st)
            wst = wpool.tile([128, 18, 128], dt.bfloat16, tag="ws1")
            nc.gpsimd.dma_start(wst[:], ws_d[1])
            ws_sb.append(wst)
            st = spool.tile([128, 31, 32], dt.bfloat16, tag="sin")
            nc.sync.dma_start(st[:], search_d[0, 1])
            sin0.append(st)
            sin_tiles[0] = sin0
            for c in range(2):
                kt = kpool.tile([128, 9, b_per, 25], dt.bfloat16, tag=f"kin{c}")
                nc.scalar.dma_start(kt[:], kin_d[c])
                kin_sb.append(kt)
                wkt = wpool.tile([128, 18, 128], dt.bfloat16, tag=f"wk{c}")
                nc.gpsimd.dma_start(wkt[:], wk_d[c])
                wk_sb.append(wkt)
            b1_sb = wpool.tile([128, 2], f32, tag="b1")
            nc.scalar.dma_start(b1_sb[:], b1_d[:])
            emit_load(1)
            eye_sb = wpool.tile([128, 128], dt.bfloat16, tag="eye")
            nc.gpsimd.dma_start(eye_sb[:], eye_d[:])
            for c in range(2):
                w1t = wpool.tile([128, 2, 128], f32r, tag=f"w1{c}")
                nc.sync.dma_start(w1t[:], w1_d[c])
                w1_sb.append(w1t)
                w2t = wpool.tile([128, 10], dt.bfloat16, tag=f"w2{c}")
                nc.sync.dma_start(w2t[:], w2_d[c])
                w2_sb.append(w2t)
            b3_sb = wpool.tile([128, 2], f32, tag="b3")
            nc.sync.dma_start(b3_sb[:], b3_d[:])
            bh_sb = wpool.tile([10, 1], f32, tag="bh")
            nc.sync.dma_start(bh_sb[:], bh_d[:])

            # ---- phase K: conv_kernel for all samples batched (N = b_per*25).
            # Emitted lazily AFTER conv_search[0] so the PE starts on conv
            # while the (larger) kin transfers are still in flight. ----
            kf_sb = []
            kfb_sb = []

            def emit_phase_k(chunks=(0, 1)):
                for cc in chunks:  # output-channel chunk
                    psk = ps_cs.tile([128, b_per, 25], f32, tag="ps")
                    n_acc = len(TAPS3) * 2
                    i = 0
                    for ci in range(2):
                        for (dy, dx) in TAPS3:
                            t2c = (dy * 3 + dx) * 2 + cc
                            nc.tensor.matmul(
                                psk[:],
                                wk_sb[ci][:, t2c, :],
                                kin_sb[ci][:, dy * 3 + dx, :, :],
                                start=(i == 0),
                                stop=(i == n_acc - 1),
                            )
                            i += 1
                    kf = kpool.tile([128, b_per, 25], f32, tag=f"kf{cc}")
                    nc.scalar.activation(kf[:], psk[:], AF.Relu, bias=b1_sb[:, cc : cc + 1])
                    kf_sb.append(kf)
                    kfb = kpool.tile([128, b_per, 25], dt.bfloat16, tag=f"kfb{cc}")
                    nc.scalar.activation(kfb[:], psk[:], AF.Relu, bias=b1_sb[:, cc : cc + 1])
                    kfb_sb.append(kfb)

            diag_tiles = {}

            def build_diag(u):
                """diag(kf[:, b, t]) for all 25 taps: dg[c, t, j] = eye[c,j]*kf[c,b,t].

                One Pool broadcast tensor_tensor (~6.5us); Pool is otherwise
                idle so this never contends with evictions or DVE chains."""
                bb, cc = u // 2, u % 2
                dg = dpool.tile([128, 25, 128], dt.bfloat16, tag="diag")
                mask = eye_sb[:].unsqueeze(1).broadcast_to([128, 25, 128])
                data = kfb_sb[cc][:, bb].unsqueeze(2).broadcast_to([128, 25, 128])
                nc.gpsimd.tensor_tensor(dg[:], mask, data, ALU.mult)
                diag_tiles[u] = dg

            sf_tiles = {}
            feat_tiles = {}
            mq = {}  # unit -> list of (acc, win, kap, first)

            def drain_mq(n):
                # round-robin across units so the per-unit DMA-accumulate RAW
                # chains interleave: the Pool SEQ never sits blocked on one
                # chain's previous add while another chain could generate
                done = 0
                while done < n and mq:
                    for u in list(mq.keys()):
                        ops = mq[u]
                        acc, win, kap, first = ops.pop(0)
                        if not ops:
                            del mq[u]
                        if first:
                            nc.scalar.activation(acc[:], win, AF.Copy, scale=kap)
                        else:
                            P = ppool.tile([128, 25, 26], f32, tag="prod")
                            nc.scalar.activation(P[:], win, AF.Copy, scale=kap)
                            nc.gpsimd.dma_start(acc[:], P[:], accum_op=ALU.add)
                        done += 1
                        if done >= n:
                            break

            def emit_convs(b, mid_hook=None):
                sin = sin_tiles.pop(b)
                sf = []
                for cc in range(2):
                    if cc == 1 and mid_hook is not None:
                        mid_hook()
                    sft = sfpool.tile([128, 29, 30], dt.bfloat16, tag="sf")
                    for (r0, nr) in CS_ROWS:
                        pscs = ps_cs.tile([128, 15, 29], f32, tag="ps")
                        n_acc = len(TAPS3) * 2
                        i = 0
                        for ci in range(2):
                            for (dy, dx) in TAPS3:
                                t2c = (dy * 3 + dx) * 2 + cc
                                nc.tensor.matmul(
                                    pscs[:, :nr, :],
                                    ws_sb[ci][:, t2c, :],
                                    sin[ci][:, dy + r0 : dy + r0 + nr, dx : dx + 29],
                                    start=(i == 0),
                                    stop=(i == n_acc - 1),
                                )
                                i += 1
                        nc.scalar.activation(
                            sft[:, r0 : r0 + nr, 0:29],
                            pscs[:, :nr, :],
                            AF.Relu,
                            bias=b2_sb[:, cc : cc + 1],
                        )
                        drain_mq(7)
                    sf.append(sft)
                sf_tiles[b] = sf

            def emit_xcorr(b):
                sf = sf_tiles.pop(b)
                feat = [None, None]
                for cc in range(2):
                    u = b * 2 + cc
                    ft = fpool.tile([128, 25, 26], f32r, tag="feat")
                    if u in M_UNITS:
                        # ACT products + DMA-accumulate: queue the per-tap ops;
                        # they are drained a few at a time between evictions
                        for ti, (dy, dx) in enumerate(TAPS5):
                            kap = kf_sb[cc][:, b, dy * 5 + dx : dy * 5 + dx + 1]
                            win = sf[cc][:, dy : dy + 25, dx : dx + 26]
                            mq.setdefault(u, []).append((ft, win, kap, ti == 0))
                        feat[cc] = ft
                        continue
                    if u in PE_UNITS:
                        dg = diag_tiles.pop(u)
                        for (r0, nr) in H_ROWS:
                            psx = ps_h.tile([128, 13, 25], f32, tag="ph")
                            for ti, (dy, dx) in enumerate(TAPS5):
                                nc.tensor.matmul(
                                    psx[:, :nr, :],
                                    dg[:, ti, :],
                                    sf[cc][:, dy + r0 : dy + r0 + nr, dx : dx + 25],
                                    start=(ti == 0),
                                    stop=(ti == 24),
                                )
                            nc.scalar.activation(
                                ft[:, r0 : r0 + nr, 0:25], psx[:, :nr, :], AF.Copy
                            )
                    else:
                        dv = ft[:, :, 0:25]
                        for ti, (dy, dx) in enumerate(TAPS5):
                            kap = kf_sb[cc][:, b, dy * 5 + dx : dy * 5 + dx + 1]
                            win = sf[cc][:, dy : dy + 25, dx : dx + 25]
                            if ti == 0:
                                nc.vector.tensor_scalar(dv, win, kap, None, ALU.mult)
                            else:
                                nc.vector.scalar_tensor_tensor(
                                    dv, win, kap, dv, ALU.mult, ALU.add
                                )
                    feat[cc] = ft

                feat_tiles[b] = feat
                drain_mq(8)

            def emit_head(b):
                feat = feat_tiles.pop(b)
                # h1: 1x1 conv + bn3 + relu -> h1o [2][128, 25, 25]
                h1o = []
                for cc2 in range(2):
                    ht = hpool.tile([128, 25, 25], dt.bfloat16, tag="h1o")
                    for (r0, nr) in H_ROWS:
                        psh = ps_h.tile([128, 13, 26], f32, tag="ph")
                        for ci in range(2):
                            nc.tensor.matmul(
                                psh[:, :nr, :],
                                w1_sb[ci][:, cc2, :],
                                feat[ci][:, r0 : r0 + nr, :],
                                start=(ci == 0),
                                stop=(ci == 1),
                            )
                        nc.scalar.activation(
                            ht[:, r0 : r0 + nr, :],
                            psh[:, :nr, 0:25],
                            AF.Relu,
                            bias=b3_sb[:, cc2 : cc2 + 1],
                        )
                    h1o.append(ht)

                # h2: 1x1 conv (+bias) -> out [10, 25, 25]
                osb = opool.tile([10, 25, 25], f32, tag="osb")
                for (r0, nr) in H_ROWS:
                    psh2 = ps_h2.tile([10, 13, 25], f32, tag="ph2")
                    for ci in range(2):
                        nc.tensor.matmul(
                            psh2[:, :nr, :],
                            w2_sb[ci][:, :],
                            h1o[ci][:, r0 : r0 + nr, :],
                            start=(ci == 0),
                            stop=(ci == 1),
                        )
                    nc.scalar.activation(
                        osb[:, r0 : r0 + nr, :],
                        psh2[:, :nr, :],
                        AF.Identity,
                        bias=bh_sb[:, :],
                    )
                # y store on the ACT ring: osb was just evicted by ACT, so the
                # DMA dispatch never waits on a semaphore
                nc.scalar.dma_start(y_d[b], osb[:])

            # ---- software-pipelined sample loop (head lags by 2 samples).
            # The head is emitted BEFORE conv_search so its PSUM evictions sit
            # ahead of conv evictions in the in-order ACT queue (no
            # head-of-line blocking -> no PE stall on ps_h banks). ----
            # diag build schedule: 2 iterations ahead of use; when a sample
            # has two PE units, stagger the second to the next iteration so
            # only 2 diag tiles are ever live (dpool bufs=2)
            diag_sched = {}
            seen_samples = set()
            for u in PE_UNITS:
                it = u // 2 - 2
                if u // 2 in seen_samples:
                    it += 1
                seen_samples.add(u // 2)
                diag_sched.setdefault(max(it, 0), []).append(u)

            for b in range(b_per):
                if b + 2 < b_per:
                    emit_load(b + 2)
                emit_convs(b, mid_hook=emit_phase_k if b == 0 else None)
                for u in diag_sched.get(b, ()):
                    build_diag(u)
                if b == b_per - 1:
                    drain_mq(10**9)
                    # tail: drain two heads before the last xcorr so their
                    # PSUM ping-pong overlaps the xcorr matmuls
                    emit_head(b - 3)
                    emit_head(b - 2)
                    emit_xcorr(b)
                    # head[15]'s feats come from the PE itself (units 30/31),
                    # so run it first while DVE drains sample 14's units
                    emit_head(b)
                    emit_head(b - 1)
                else:
                    emit_xcorr(b)
                    if b >= 3:
                        emit_head(b - 3)

    nc.compile()
    return nc


def _get_nc(b_per=B_PER):
    key = b_per
    if key not in _NC_CACHE:
        _NC_CACHE[key] = _build_nc(b_per)
    return _NC_CACHE[key]


def _host_prep(inputs):
    """Fold BN into weights, transpose to lhsT layouts, slice per core."""
    import ml_dtypes

    bf16 = ml_dtypes.bfloat16
    f = np.float32
    kernel = np.ascontiguousarray(inputs["kernel"], dtype=f)
    search = np.ascontiguousarray(inputs["search"], dtype=f)

    def bn_fold(g, b_, m, v):
        scale = g / np.sqrt(v + EPS)
        shift = b_ - m * scale
        return scale.astype(f), shift.astype(f)

    s1, sh1 = bn_fold(inputs["g1"], inputs["b1"], inputs["m1"], inputs["v1"])
    s2, sh2 = bn_fold(inputs["g2"], inputs["b2"], inputs["m2"], inputs["v2"])
    s3, sh3 = bn_fold(inputs["g3"], inputs["b3"], inputs["m3"], inputs["v3"])

    def conv3_lhsT(w, scale):
        # w [co=256, ci=256, 3, 3] * scale[co] -> [cic, ci128, tap*2+coc, co128]
        wf = (w * scale[:, None, None, None]).astype(f)
        wf = wf.reshape(2, 128, 2, 128, 3, 3)  # [coc, co, cic, ci, dy, dx]
        wf = wf.transpose(2, 3, 4, 5, 0, 1)  # [cic, ci, dy, dx, coc, co]
        return np.ascontiguousarray(wf.reshape(2, 128, 18, 128))

    wk = conv3_lhsT(inputs["w_ck"], s1).astype(bf16)
    ws = conv3_lhsT(inputs["w_cs"], s2).astype(bf16)

    w1 = (inputs["w_h1"][:, :, 0, 0] * s3[:, None]).astype(f)  # [co 256, ci 256]
    w1 = w1.reshape(2, 128, 2, 128).transpose(2, 3, 0, 1)  # [cic, ci, coc, co]
    w1 = np.ascontiguousarray(w1)
    w2 = inputs["w_h2"][:, :, 0, 0].astype(f)  # [10, 256]
    w2 = np.ascontiguousarray(w2.reshape(10, 2, 128).transpose(1, 2, 0)).astype(bf16)  # [cic, ci, 10]

    weights = dict(
        wk=wk,
        ws=ws,
        w1=w1,
        w2=w2,
        eye=np.eye(128, dtype=bf16),
        b1s=np.ascontiguousarray(sh1.reshape(2, 128).T),
        b2s=np.ascontiguousarray(sh2.reshape(2, 128).T),
        b3s=np.ascontiguousarray(sh3.reshape(2, 128).T),
        bhs=np.ascontiguousarray(inputs["b_h2"].astype(f).reshape(10, 1)),
    )

    in_maps = []
    for c in range(N_CORES):
        sl = slice(c * B_PER, (c + 1) * B_PER)
        win = np.lib.stride_tricks.sliding_window_view(kernel[sl], (5, 5), axis=(2, 3))
        # win[b, c, dy, dx, y, x] = kernel[b, c, y+dy, x+dx]
        kin = win.reshape(B_PER, 2, 128, 9, 25).transpose(1, 2, 3, 0, 4)
        sp = np.zeros((B_PER, 2, 128, 31, 32), dtype=bf16)
        sp[..., :31] = search[sl].reshape(B_PER, 2, 128, 31, 31).astype(bf16)
        m = dict(weights)
        m["search"] = sp
        m["kin"] = np.ascontiguousarray(kin).astype(bf16)
        in_maps.append(m)
    return in_maps


def run(trace=False, **inputs):
    from concourse import bass_utils

    in_maps = _host_prep(inputs)
    nc = _get_nc()
    try:
        res = bass_utils.run_bass_kernel_spmd(
            nc, in_maps, core_ids=list(range(N_CORES)), trace=trace
        )
    except ModuleNotFoundError:
        # NTFF profiling hook unavailable in this container
        res = bass_utils.run_bass_kernel_spmd(
            nc, in_maps, core_ids=list(range(N_CORES)), trace=False
        )
    y = np.concatenate([res.results[c]["y"] for c in range(N_CORES)], axis=0)
    return y.reshape(B, 10, 25, 25), res


def kernel(**inputs):
    y, _ = run(trace=False, **inputs)
    return y


# revision 72
# speedup vs baseline: 1.0166x; 1.0166x over previous
"""Trainium2 Bass kernel for nn_DepthwiseXCorr (SiamRPN-style depthwise-xcorr head).

Pipeline per sample (B=128 sharded 16/core across 8 cores, pure data parallel):
  k = relu(bn1(conv3x3(kernel_in, w_ck)))      [256, 5, 5]
  s = relu(bn2(conv3x3(search_in, w_cs)))      [256, 29, 29]
  feat = depthwise_xcorr(s, k)                 [256, 25, 25]
  h = relu(bn3(conv1x1(feat, w_h1)))           [256, 25, 25]
  out = conv1x1(h, w_h2) + b_h2                [10, 25, 25]

Implementation notes:
  - BN scale folded into conv weights host-side; BN shift + ReLU applied by
    the ACT engine on the PSUM->SBUF eviction (activation = relu(x*1 + bias)).
  - Convs are implicit GEMM on TensorE.  conv_search runs in bf16 (1 cyc/row,
    same PE rate as fp32r, but exempt from the fp32r even-innermost-run ISA
    rule, so the PSUM tiles are 29 wide instead of 30) with fp32 PSUM
    accumulation; conv_kernel inputs (kin im2col + wk) are bf16 to halve
    their startup DMA time.
  - Depthwise xcorr (32 (sample,chunk) units/core) is split across the two
    engines that can do per-partition-scalar MACs at full rate: 14 units run
    on the PE as diagonal-weight matmuls (25 matmuls/unit, diag tiles built
    by Pool broadcast tensor_tensor one sample ahead); 18 units run as
    25-tap scalar_tensor_tensor chains on DVE (bf16 windows, fp32
    accumulator).  GPSIMD supports no scalar_tensor_tensor on real HW, and
    ACT-product + gpsimd DMA-accumulate lanes lose to SWDGE descriptor-gen
    cost, so both stay out of the steady state (M_UNITS kept for reference).
  - The head (h1/h2) is software-pipelined 3 samples behind conv/xcorr so
    DVE's unit lag never stalls the PE (the PE p-state drops on any stall);
    the tail drains two heads before the last sample's xcorr so their PSUM
    ping-pong overlaps xcorr matmuls.
  - DMA discipline: transfers serialize on the DMA engines, so startup loads
    are ordered strictly by first use; search tiles are host-padded to
    [31,32] so each load is one descriptor per partition; y stores issue on
    the ACT ring right after their eviction so they never wait.
"""

import numpy as np

EPS = 1e-5
N_CORES = 8
B = 128
B_PER = B // N_CORES  # 16
CIN = 256
H = 256
COUT = 10

_NC_CACHE = {}

# unit u = sample*2 + chunk.  Three xcorr lanes:
#   P: PE diagonal-weight matmuls (diag tiles built by Pool broadcasts)
#   M: ACT per-partition-scaled products + gpsimd DMA-accumulate chains
#   V: DVE 25-tap scalar_tensor_tensor chains
# GPSIMD itself supports no scalar_tensor_tensor on real HW.
PE_UNITS = (5, 7, 9, 11, 13, 15, 17, 19, 21, 23, 25, 27, 30, 31)
M_UNITS = ()


def _build_nc(b_per=B_PER):
    """Build the Bass program for one core processing `b_per` samples."""
    import concourse.bacc as bacc
    import concourse.mybir as mybir
    import concourse.tile as tile

    dt = mybir.dt
    f32 = dt.float32
    f32r = dt.float32r
    AF = mybir.ActivationFunctionType
    ALU = mybir.AluOpType

    nc = bacc.Bacc("TRN2", target_bir_lowering=False, debug=False)

    # ---- DRAM tensors (shapes match SBUF tiles exactly; host pre-transposes) ----
    search_d = nc.dram_tensor("search", [b_per, 2, 128, 31, 32], dt.bfloat16, kind="ExternalInput")
    kin_d = nc.dram_tensor("kin", [2, 128, 9, b_per, 25], dt.bfloat16, kind="ExternalInput")
    wk_d = nc.dram_tensor("wk", [2, 128, 18, 128], dt.bfloat16, kind="ExternalInput")
    ws_d = nc.dram_tensor("ws", [2, 128, 18, 128], dt.bfloat16, kind="ExternalInput")
    w1_d = nc.dram_tensor("w1", [2, 128, 2, 128], f32r, kind="ExternalInput")
    w2_d = nc.dram_tensor("w2", [2, 128, 10], dt.bfloat16, kind="ExternalInput")
    eye_d = nc.dram_tensor("eye", [128, 128], dt.bfloat16, kind="ExternalInput")
    b1_d = nc.dram_tensor("b1s", [128, 2], f32, kind="ExternalInput")
    b2_d = nc.dram_tensor("b2s", [128, 2], f32, kind="ExternalInput")
    b3_d = nc.dram_tensor("b3s", [128, 2], f32, kind="ExternalInput")
    bh_d = nc.dram_tensor("bhs", [10, 1], f32, kind="ExternalInput")
    y_d = nc.dram_tensor("y", [b_per, 10, 25, 25], f32, kind="ExternalOutput")

    TAPS3 = [(dy, dx) for dy in range(3) for dx in range(3)]
    TAPS5 = [(dy, dx) for dy in range(5) for dx in range(5)]
    # conv_search output row tiling: 29 rows -> two PSUM tiles (N = 435 / 406)
    CS_ROWS = [(0, 15), (15, 14)]
    # h1/h2/PE-xcorr output row tiling: 25 rows -> two PSUM tiles (N = 325 / 300)
    H_ROWS = [(0, 13), (13, 12)]

    with tile.TileContext(nc) as tc:
        with (
            tc.tile_pool(name="wpool", bufs=1) as wpool,
            tc.tile_pool(name="kpool", bufs=1) as kpool,
            tc.tile_pool(name="spool", bufs=8) as spool,
            tc.tile_pool(name="fpool", bufs=12) as fpool,
            tc.tile_pool(name="hpool", bufs=5) as hpool,
            tc.tile_pool(name="sfpool", bufs=8) as sfpool,
            tc.tile_pool(name="opool", bufs=3) as opool,
            tc.tile_pool(name="dpool", bufs=3) as dpool,
            tc.tile_pool(name="ppool", bufs=5) as ppool,
            tc.tile_pool(name="ps_cs", bufs=2, space="PSUM") as ps_cs,
            tc.tile_pool(name="ps_h", bufs=4, space="PSUM") as ps_h,
            tc.tile_pool(name="ps_h2", bufs=2, space="PSUM") as ps_h2,
        ):
            sin_tiles = {}

            def emit_load(b):
                sin = []
                for ci in range(2):
                    st = spool.tile([128, 31, 32], dt.bfloat16, tag="sin")
                    nc.sync.dma_start(st[:], search_d[b, ci])
                    sin.append(st)
                sin_tiles[b] = sin

            # ---- startup DMA schedule (ACT ring: kin + biases; Pool ring:
            # ws + wk + eye; SP ring: search samples + head weights) ----
            kin_sb = []
            wk_sb = []
            ws_sb = []
            w1_sb = []
            w2_sb = []
            # transfers serialize on the DMA engines, so order strictly by
            # first use: conv_search[0] inputs, then phase-K inputs, then the
            # second sample and head weights
            b2_sb = wpool.tile([128, 2], f32, tag="b2")
            nc.scalar.dma_start(b2_sb[:], b2_d[:])
            # interleave so conv_search[0]'s first chunk (ws0 + sin0c0) heads
            # the serialized DMA queue
            sin0 = []
            wst = wpool.tile([128, 18, 128], dt.bfloat16, tag="ws0")
            nc.gpsimd.dma_start(wst[:], ws_d[0])
            ws_sb.append(wst)
            st = spool.tile([128, 31, 32], dt.bfloat16, tag="sin")
            nc.sync.dma_start(st[:], search_d[0, 0])
            sin0.append(st)
            wst = wpool.tile([128, 18, 128], dt.bfloat16, tag="ws1")
            nc.gpsimd.dma_start(wst[:], ws_d[1])
            ws_sb.append(wst)
            st = spool.tile([128, 31, 32], dt.bfloat16, tag="sin")
            nc.sync.dma_start(st[:], search_d[0, 1])
            sin0.append(st)
            sin_tiles[0] = sin0
            for c in range(2):
                kt = kpool.tile([128, 9, b_per, 25], dt.bfloat16, tag=f"kin{c}")
                nc.scalar.dma_start(kt[:], kin_d[c])
                kin_sb.append(kt)
                wkt = wpool.tile([128, 18, 128], dt.bfloat16, tag=f"wk{c}")
                nc.gpsimd.dma_start(wkt[:], wk_d[c])
                wk_sb.append(wkt)
            b1_sb = wpool.tile([128, 2], f32, tag="b1")
            nc.scalar.dma_start(b1_sb[:], b1_d[:])
            emit_load(1)
            eye_sb = wpool.tile([128, 128], dt.bfloat16, tag="eye")
            nc.gpsimd.dma_start(eye_sb[:], eye_d[:])
            for c in range(2):
                w1t = wpool.tile([128, 2, 128], f32r, tag=f"w1{c}")
                nc.sync.dma_start(w1t[:], w1_d[c])
                w1_sb.append(w1t)
                w2t = wpool.tile([128, 10], dt.bfloat16, tag=f"w2{c}")
                nc.sync.dma_start(w2t[:], w2_d[c])
                w2_sb.append(w2t)
            b3_sb = wpool.tile([128, 2], f32, tag="b3")
            nc.sync.dma_start(b3_sb[:], b3_d[:])
            bh_sb = wpool.tile([10, 1], f32, tag="bh")
            nc.sync.dma_start(bh_sb[:], bh_d[:])

            # ---- phase K: conv_kernel for all samples batched (N = b_per*25).
            # Emitted lazily AFTER conv_search[0] so the PE starts on conv
            # while the (larger) kin transfers are still in flight. ----
            kf_sb = []
            kfb_sb = []

            def emit_phase_k(chunks=(0, 1)):
                for cc in chunks:  # output-channel chunk
                    psk = ps_cs.tile([128, b_per, 25], f32, tag="ps")
                    n_acc = len(TAPS3) * 2
                    i = 0
                    for ci in range(2):
                        for (dy, dx) in TAPS3:
                            t2c = (dy * 3 + dx) * 2 + cc
                            nc.tensor.matmul(
                                psk[:],
                                wk_sb[ci][:, t2c, :],
                                kin_sb[ci][:, dy * 3 + dx, :, :],
                                start=(i == 0),
                                stop=(i == n_acc - 1),
                            )
                            i += 1
                    kf = kpool.tile([128, b_per, 25], f32, tag=f"kf{cc}")
                    nc.scalar.activation(kf[:], psk[:], AF.Relu, bias=b1_sb[:, cc : cc + 1])
                    kf_sb.append(kf)
                    kfb = kpool.tile([128, b_per, 25], dt.bfloat16, tag=f"kfb{cc}")
                    nc.scalar.activation(kfb[:], psk[:], AF.Relu, bias=b1_sb[:, cc : cc + 1])
                    kfb_sb.append(kfb)

            diag_tiles = {}

            def build_diag(u):
                """diag(kf[:, b, t]) for all 25 taps: dg[c, t, j] = eye[c,j]*kf[c,b,t].

                One Pool broadcast tensor_tensor (~6.5us); Pool is otherwise
                idle so this never contends with evictions or DVE chains."""
                bb, cc = u // 2, u % 2
                dg = dpool.tile([128, 25, 128], dt.bfloat16, tag="diag")
                mask = eye_sb[:].unsqueeze(1).broadcast_to([128, 25, 128])
                data = kfb_sb[cc][:, bb].unsqueeze(2).broadcast_to([128, 25, 128])
                nc.gpsimd.tensor_tensor(dg[:], mask, data, ALU.mult)
                diag_tiles[u] = dg

            sf_tiles = {}
            feat_tiles = {}
            mq = {}  # unit -> list of (acc, win, kap, first)

            def drain_mq(n):
                # round-robin across units so the per-unit DMA-accumulate RAW
                # chains interleave: the Pool SEQ never sits blocked on one
                # chain's previous add while another chain could generate
                done = 0
                while done < n and mq:
                    for u in list(mq.keys()):
                        ops = mq[u]
                        acc, win, kap, first = ops.pop(0)
                        if not ops:
                            del mq[u]
                        if first:
                            nc.scalar.activation(acc[:], win, AF.Copy, scale=kap)
                        else:
                            P = ppool.tile([128, 25, 26], f32, tag="prod")
                            nc.scalar.activation(P[:], win, AF.Copy, scale=kap)
                            nc.gpsimd.dma_start(acc[:], P[:], accum_op=ALU.add)
                        done += 1
                        if done >= n:
                            break

            def emit_convs(b, mid_hook=None):
                sin = sin_tiles.pop(b)
                sf = []
                for cc in range(2):
                    if cc == 1 and mid_hook is not None:
                        mid_hook()
                    sft = sfpool.tile([128, 29, 30], dt.bfloat16, tag="sf")
                    for (r0, nr) in CS_ROWS:
                        pscs = ps_cs.tile([128, 15, 29], f32, tag="ps")
                        n_acc = len(TAPS3) * 2
                        i = 0
                        for ci in range(2):
                            for (dy, dx) in TAPS3:
                                t2c = (dy * 3 + dx) * 2 + cc
                                nc.tensor.matmul(
                                    pscs[:, :nr, :],
                                    ws_sb[ci][:, t2c, :],
                                    sin[ci][:, dy + r0 : dy + r0 + nr, dx : dx + 29],
                                    start=(i == 0),
                                    stop=(i == n_acc - 1),
                                )
                                i += 1
                        nc.scalar.activation(
                            sft[:, r0 : r0 + nr, 0:29],
                            pscs[:, :nr, :],
                            AF.Relu,
                            bias=b2_sb[:, cc : cc + 1],
                        )
                        drain_mq(7)
                    sf.append(sft)
                sf_tiles[b] = sf

            def emit_xcorr(b):
                sf = sf_tiles.pop(b)
                feat = [None, None]
                for cc in range(2):
                    u = b * 2 + cc
                    ft = fpool.tile([128, 25, 26], f32r, tag="feat")
                    if u in M_UNITS:
                        # ACT products + DMA-accumulate: queue the per-tap ops;
                        # they are drained a few at a time between evictions
                        for ti, (dy, dx) in enumerate(TAPS5):
                            kap = kf_sb[cc][:, b, dy * 5 + dx : dy * 5 + dx + 1]
                            win = sf[cc][:, dy : dy + 25, dx : dx + 26]
                            mq.setdefault(u, []).append((ft, win, kap, ti == 0))
                        feat[cc] = ft
                        continue
                    if u in PE_UNITS:
                        dg = diag_tiles.pop(u)
                        for (r0, nr) in H_ROWS:
                            psx = ps_h.tile([128, 13, 25], f32, tag="ph")
                            for ti, (dy, dx) in enumerate(TAPS5):
                                nc.tensor.matmul(
                                    psx[:, :nr, :],
                                    dg[:, ti, :],
                                    sf[cc][:, dy + r0 : dy + r0 + nr, dx : dx + 25],
                                    start=(ti == 0),
                                    stop=(ti == 24),
                                )
                            nc.scalar.activation(
                                ft[:, r0 : r0 + nr, 0:25], psx[:, :nr, :], AF.Copy
                            )
                    else:
                        dv = ft[:, :, 0:25]
                        for ti, (dy, dx) in enumerate(TAPS5):
                            kap = kf_sb[cc][:, b, dy * 5 + dx : dy * 5 + dx + 1]
                            win = sf[cc][:, dy : dy + 25, dx : dx + 25]
                            if ti == 0:
                                nc.vector.tensor_scalar(dv, win, kap, None, ALU.mult)
                            else:
                                nc.vector.scalar_tensor_tensor(
                                    dv, win, kap, dv, ALU.mult, ALU.add
                                )
                    feat[cc] = ft

                feat_tiles[b] = feat
                drain_mq(8)

            def emit_head(b):
                feat = feat_tiles.pop(b)
                # h1: 1x1 conv + bn3 + relu -> h1o [2][128, 25, 25]
                h1o = []
                for cc2 in range(2):
                    ht = hpool.tile([128, 25, 25], dt.bfloat16, tag="h1o")
                    for (r0, nr) in H_ROWS:
                        psh = ps_h.tile([128, 13, 26], f32, tag="ph")
                        for ci in range(2):
                            nc.tensor.matmul(
                                psh[:, :nr, :],
                                w1_sb[ci][:, cc2, :],
                                feat[ci][:, r0 : r0 + nr, :],
                                start=(ci == 0),
                                stop=(ci == 1),
                            )
                        nc.scalar.activation(
                            ht[:, r0 : r0 + nr, :],
                            psh[:, :nr, 0:25],
                            AF.Relu,
                            bias=b3_sb[:, cc2 : cc2 + 1],
                        )
                    h1o.append(ht)

                # h2: 1x1 conv (+bias) -> out [10, 25, 25]
                osb = opool.tile([10, 25, 25], f32, tag="osb")
                for (r0, nr) in H_ROWS:
                    psh2 = ps_h2.tile([10, 13, 25], f32, tag="ph2")
                    for ci in range(2):
                        nc.tensor.matmul(
                            psh2[:, :nr, :],
                            w2_sb[ci][:, :],
                            h1o[ci][:, r0 : r0 + nr, :],
                            start=(ci == 0),
                            stop=(ci == 1),
                        )
                    nc.scalar.activation(
                        osb[:, r0 : r0 + nr, :],
                        psh2[:, :nr, :],
                        AF.Identity,
                        bias=bh_sb[:, :],
                    )
                # y store on the ACT ring: osb was just evicted by ACT, so the
                # DMA dispatch never waits on a semaphore
                nc.scalar.dma_start(y_d[b], osb[:])

            # ---- software-pipelined sample loop (head lags by 2 samples).
            # The head is emitted BEFORE conv_search so its PSUM evictions sit
            # ahead of conv evictions in the in-order ACT queue (no
            # head-of-line blocking -> no PE stall on ps_h banks). ----
            # diag build schedule: 2 iterations ahead of use; when a sample
            # has two PE units, stagger the second to the next iteration so
            # only 2 diag tiles are ever live (dpool bufs=2)
            diag_sched = {}
            seen_samples = set()
            for u in PE_UNITS:
                it = u // 2 - 2
                if u // 2 in seen_samples:
                    it += 1
                seen_samples.add(u // 2)
                diag_sched.setdefault(max(it, 0), []).append(u)

            for b in range(b_per):
                if b + 2 < b_per:
                    emit_load(b + 2)
                emit_convs(b, mid_hook=emit_phase_k if b == 0 else None)
                for u in diag_sched.get(b, ()):
                    build_diag(u)
                if b == b_per - 1:
                    drain_mq(10**9)
                    # tail: drain two heads before the last xcorr so their
                    # PSUM ping-pong overlaps the xcorr matmuls
                    emit_head(b - 3)
                    emit_head(b - 2)
                    emit_xcorr(b)
                    # head[15]'s feats come from the PE itself (units 30/31),
                    # so run it first while DVE drains sample 14's units
                    emit_head(b)
                    emit_head(b - 1)
                else:
                    emit_xcorr(b)
                    if b >= 3:
                        emit_head(b - 3)

    nc.compile()
    return nc


def _get_nc(b_per=B_PER):
    key = b_per
    if key not in _NC_CACHE:
        _NC_CACHE[key] = _build_nc(b_per)
    return _NC_CACHE[key]


def _host_prep(inputs):
    """Fold BN into weights, transpose to lhsT layouts, slice per core."""
    import ml_dtypes

    bf16 = ml_dtypes.bfloat16
    f = np.float32
    kernel = np.ascontiguousarray(inputs["kernel"], dtype=f)
    search = np.ascontiguousarray(inputs["search"], dtype=f)

    def bn_fold(g, b_, m, v):
        scale = g / np.sqrt(v + EPS)
        shift = b_ - m * scale
        return scale.astype(f), shift.astype(f)

    s1, sh1 = bn_fold(inputs["g1"], inputs["b1"], inputs["m1"], inputs["v1"])
    s2, sh2 = bn_fold(inputs["g2"], inputs["b2"], inputs["m2"], inputs["v2"])
    s3, sh3 = bn_fold(inputs["g3"], inputs["b3"], inputs["m3"], inputs["v3"])

    def conv3_lhsT(w, scale):
        # w [co=256, ci=256, 3, 3] * scale[co] -> [cic, ci128, tap*2+coc, co128]
        wf = (w * scale[:, None, None, None]).astype(f)
        wf = wf.reshape(2, 128, 2, 128, 3, 3)  # [coc, co, cic, ci, dy, dx]
        wf = wf.transpose(2, 3, 4, 5, 0, 1)  # [cic, ci, dy, dx, coc, co]
        return np.ascontiguousarray(wf.reshape(2, 128, 18, 128))

    wk = conv3_lhsT(inputs["w_ck"], s1).astype(bf16)
    ws = conv3_lhsT(inputs["w_cs"], s2).astype(bf16)

    w1 = (inputs["w_h1"][:, :, 0, 0] * s3[:, None]).astype(f)  # [co 256, ci 256]
    w1 = w1.reshape(2, 128, 2, 128).transpose(2, 3, 0, 1)  # [cic, ci, coc, co]
    w1 = np.ascontiguousarray(w1)
    w2 = inputs["w_h2"][:, :, 0, 0].astype(f)  # [10, 256]
    w2 = np.ascontiguousarray(w2.reshape(10, 2, 128).transpose(1, 2, 0)).astype(bf16)  # [cic, ci, 10]

    weights = dict(
        wk=wk,
        ws=ws,
        w1=w1,
        w2=w2,
        eye=np.eye(128, dtype=bf16),
        b1s=np.ascontiguousarray(sh1.reshape(2, 128).T),
        b2s=np.ascontiguousarray(sh2.reshape(2, 128).T),
        b3s=np.ascontiguousarray(sh3.reshape(2, 128).T),
        bhs=np.ascontiguousarray(inputs["b_h2"].astype(f).reshape(10, 1)),
    )

    in_maps = []
    for c in range(N_CORES):
        sl = slice(c * B_PER, (c + 1) * B_PER)
        win = np.lib.stride_tricks.sliding_window_view(kernel[sl], (5, 5), axis=(2, 3))
        # win[b, c, dy, dx, y, x] = kernel[b, c, y+dy, x+dx]
        kin = win.reshape(B_PER, 2, 128, 9, 25).transpose(1, 2, 3, 0, 4)
        sp = np.zeros((B_PER, 2, 128, 31, 32), dtype=bf16)
        sp[..., :31] = search[sl].reshape(B_PER, 2, 128, 31, 31).astype(bf16)
        m = dict(weights)
        m["search"] = sp
        m["kin"] = np.ascontiguousarray(kin).astype(bf16)
        in_maps.append(m)
    return in_maps


def run(trace=False, **inputs):
    from concourse import bass_utils

    in_maps = _host_prep(inputs)
    nc = _get_nc()
    try:
        res = bass_utils.run_bass_kernel_spmd(
            nc, in_maps, core_ids=list(range(N_CORES)), trace=trace
        )
    except ModuleNotFoundError:
        # NTFF profiling hook unavailable in this container
        res = bass_utils.run_bass_kernel_spmd(
            nc, in_maps, core_ids=list(range(N_CORES)), trace=False
        )
    y = np.concatenate([res.results[c]["y"] for c in range(N_CORES)], axis=0)
    return y.reshape(B, 10, 25, 25), res


def kernel(**inputs):
    y, _ = run(trace=False, **inputs)
    return y


# revision 73
# speedup vs baseline: 1.0177x; 1.0011x over previous
"""Trainium2 Bass kernel for nn_DepthwiseXCorr (SiamRPN-style depthwise-xcorr head).

Pipeline per sample (B=128 sharded 16/core across 8 cores, pure data parallel):
  k = relu(bn1(conv3x3(kernel_in, w_ck)))      [256, 5, 5]
  s = relu(bn2(conv3x3(search_in, w_cs)))      [256, 29, 29]
  feat = depthwise_xcorr(s, k)                 [256, 25, 25]
  h = relu(bn3(conv1x1(feat, w_h1)))           [256, 25, 25]
  out = conv1x1(h, w_h2) + b_h2                [10, 25, 25]

Implementation notes:
  - BN scale folded into conv weights host-side; BN shift + ReLU applied by
    the ACT engine on the PSUM->SBUF eviction (activation = relu(x*1 + bias)).
  - Convs are implicit GEMM on TensorE.  conv_search runs in bf16 (1 cyc/row,
    same PE rate as fp32r, but exempt from the fp32r even-innermost-run ISA
    rule, so the PSUM tiles are 29 wide instead of 30) with fp32 PSUM
    accumulation; conv_kernel inputs (kin im2col + wk) are bf16 to halve
    their startup DMA time.
  - Depthwise xcorr (32 (sample,chunk) units/core) is split across the two
    engines that can do per-partition-scalar MACs at full rate: 14 units run
    on the PE as diagonal-weight matmuls (25 matmuls/unit, diag tiles built
    by Pool broadcast tensor_tensor one sample ahead); 18 units run as
    25-tap scalar_tensor_tensor chains on DVE (bf16 windows, fp32
    accumulator).  GPSIMD supports no scalar_tensor_tensor on real HW, and
    ACT-product + gpsimd DMA-accumulate lanes lose to SWDGE descriptor-gen
    cost, so both stay out of the steady state (M_UNITS kept for reference).
  - The head (h1/h2) is software-pipelined 3 samples behind conv/xcorr so
    DVE's unit lag never stalls the PE (the PE p-state drops on any stall);
    the tail drains two heads before the last sample's xcorr so their PSUM
    ping-pong overlaps xcorr matmuls.
  - DMA discipline: transfers serialize on the DMA engines, so startup loads
    are ordered strictly by first use; search tiles are host-padded to
    [31,32] so each load is one descriptor per partition; y stores issue on
    the ACT ring right after their eviction so they never wait.
"""

import numpy as np

EPS = 1e-5
N_CORES = 8
B = 128
B_PER = B // N_CORES  # 16
CIN = 256
H = 256
COUT = 10

_NC_CACHE = {}

# unit u = sample*2 + chunk.  Three xcorr lanes:
#   P: PE diagonal-weight matmuls (diag tiles built by Pool broadcasts)
#   M: ACT per-partition-scaled products + gpsimd DMA-accumulate chains
#   V: DVE 25-tap scalar_tensor_tensor chains
# GPSIMD itself supports no scalar_tensor_tensor on real HW.
PE_UNITS = (5, 7, 9, 11, 13, 15, 17, 19, 21, 23, 25, 27, 30, 31)
M_UNITS = ()


def _build_nc(b_per=B_PER):
    """Build the Bass program for one core processing `b_per` samples."""
    import concourse.bacc as bacc
    import concourse.mybir as mybir
    import concourse.tile as tile

    dt = mybir.dt
    f32 = dt.float32
    f32r = dt.float32r
    AF = mybir.ActivationFunctionType
    ALU = mybir.AluOpType

    nc = bacc.Bacc("TRN2", target_bir_lowering=False, debug=False)

    # ---- DRAM tensors (shapes match SBUF tiles exactly; host pre-transposes) ----
    search_d = nc.dram_tensor("search", [b_per, 2, 128, 31, 32], dt.bfloat16, kind="ExternalInput")
    kin_d = nc.dram_tensor("kin", [2, 128, 9, b_per, 25], dt.bfloat16, kind="ExternalInput")
    wk_d = nc.dram_tensor("wk", [2, 128, 18, 128], dt.bfloat16, kind="ExternalInput")
    ws_d = nc.dram_tensor("ws", [2, 128, 18, 128], dt.bfloat16, kind="ExternalInput")
    w1_d = nc.dram_tensor("w1", [2, 128, 2, 128], f32r, kind="ExternalInput")
    w2_d = nc.dram_tensor("w2", [2, 128, 10], dt.bfloat16, kind="ExternalInput")
    eye_d = nc.dram_tensor("eye", [128, 128], dt.bfloat16, kind="ExternalInput")
    b1_d = nc.dram_tensor("b1s", [128, 2], f32, kind="ExternalInput")
    b2_d = nc.dram_tensor("b2s", [128, 2], f32, kind="ExternalInput")
    b3_d = nc.dram_tensor("b3s", [128, 2], f32, kind="ExternalInput")
    bh_d = nc.dram_tensor("bhs", [10, 1], f32, kind="ExternalInput")
    y_d = nc.dram_tensor("y", [b_per, 10, 25, 25], f32, kind="ExternalOutput")

    TAPS3 = [(dy, dx) for dy in range(3) for dx in range(3)]
    TAPS5 = [(dy, dx) for dy in range(5) for dx in range(5)]
    # conv_search output row tiling: 29 rows -> two PSUM tiles (N = 435 / 406)
    CS_ROWS = [(0, 15), (15, 14)]
    # h1/h2/PE-xcorr output row tiling: 25 rows -> two PSUM tiles (N = 325 / 300)
    H_ROWS = [(0, 13), (13, 12)]

    with tile.TileContext(nc) as tc:
        with (
            tc.tile_pool(name="wpool", bufs=1) as wpool,
            tc.tile_pool(name="kpool", bufs=1) as kpool,
            tc.tile_pool(name="spool", bufs=8) as spool,
            tc.tile_pool(name="fpool", bufs=12) as fpool,
            tc.tile_pool(name="hpool", bufs=5) as hpool,
            tc.tile_pool(name="sfpool", bufs=8) as sfpool,
            tc.tile_pool(name="opool", bufs=3) as opool,
            tc.tile_pool(name="dpool", bufs=3) as dpool,
            tc.tile_pool(name="ppool", bufs=5) as ppool,
            tc.tile_pool(name="ps_cs", bufs=2, space="PSUM") as ps_cs,
            tc.tile_pool(name="ps_h", bufs=4, space="PSUM") as ps_h,
            tc.tile_pool(name="ps_h2", bufs=2, space="PSUM") as ps_h2,
        ):
            sin_tiles = {}

            def emit_load(b):
                sin = []
                for ci in range(2):
                    st = spool.tile([128, 31, 32], dt.bfloat16, tag="sin")
                    nc.sync.dma_start(st[:], search_d[b, ci])
                    sin.append(st)
                sin_tiles[b] = sin

            # ---- startup DMA schedule (ACT ring: kin + biases; Pool ring:
            # ws + wk + eye; SP ring: search samples + head weights) ----
            kin_sb = []
            wk_sb = []
            ws_sb = []
            w1_sb = []
            w2_sb = []
            # transfers serialize on the DMA engines, so order strictly by
            # first use: conv_search[0] inputs, then phase-K inputs, then the
            # second sample and head weights
            b2_sb = wpool.tile([128, 2], f32, tag="b2")
            nc.scalar.dma_start(b2_sb[:], b2_d[:])
            # interleave so conv_search[0]'s first chunk (ws0 + sin0c0) heads
            # the serialized DMA queue
            sin0 = []
            wst = wpool.tile([128, 18, 128], dt.bfloat16, tag="ws0")
            nc.gpsimd.dma_start(wst[:], ws_d[0])
            ws_sb.append(wst)
            st = spool.tile([128, 31, 32], dt.bfloat16, tag="sin")
            nc.sync.dma_start(st[:], search_d[0, 0])
            sin0.append(st)
            wst = wpool.tile([128, 18, 128], dt.bfloat16, tag="ws1")
            nc.gpsimd.dma_start(wst[:], ws_d[1])
            ws_sb.append(wst)
            st = spool.tile([128, 31, 32], dt.bfloat16, tag="sin")
            nc.sync.dma_start(st[:], search_d[0, 1])
            sin0.append(st)
            sin_tiles[0] = sin0
            for c in range(2):
                kt = kpool.tile([128, 9, b_per, 25], dt.bfloat16, tag=f"kin{c}")
                nc.scalar.dma_start(kt[:], kin_d[c])
                kin_sb.append(kt)
                wkt = wpool.tile([128, 18, 128], dt.bfloat16, tag=f"wk{c}")
                nc.gpsimd.dma_start(wkt[:], wk_d[c])
                wk_sb.append(wkt)
            b1_sb = wpool.tile([128, 2], f32, tag="b1")
            nc.scalar.dma_start(b1_sb[:], b1_d[:])
            emit_load(1)
            eye_sb = wpool.tile([128, 128], dt.bfloat16, tag="eye")
            nc.gpsimd.dma_start(eye_sb[:], eye_d[:])
            for c in range(2):
                w1t = wpool.tile([128, 2, 128], f32r, tag=f"w1{c}")
                nc.sync.dma_start(w1t[:], w1_d[c])
                w1_sb.append(w1t)
                w2t = wpool.tile([128, 10], dt.bfloat16, tag=f"w2{c}")
                nc.sync.dma_start(w2t[:], w2_d[c])
                w2_sb.append(w2t)
            b3_sb = wpool.tile([128, 2], f32, tag="b3")
            nc.sync.dma_start(b3_sb[:], b3_d[:])
            bh_sb = wpool.tile([10, 1], f32, tag="bh")
            nc.sync.dma_start(bh_sb[:], bh_d[:])

            # ---- phase K: conv_kernel for all samples batched (N = b_per*25).
            # Emitted lazily AFTER conv_search[0] so the PE starts on conv
            # while the (larger) kin transfers are still in flight. ----
            kf_sb = []
            kfb_sb = []

            def emit_phase_k(chunks=(0, 1)):
                for cc in chunks:  # output-channel chunk
                    psk = ps_h.tile([128, b_per, 25], f32, tag="ph")
                    n_acc = len(TAPS3) * 2
                    i = 0
                    for ci in range(2):
                        for (dy, dx) in TAPS3:
                            t2c = (dy * 3 + dx) * 2 + cc
                            nc.tensor.matmul(
                                psk[:],
                                wk_sb[ci][:, t2c, :],
                                kin_sb[ci][:, dy * 3 + dx, :, :],
                                start=(i == 0),
                                stop=(i == n_acc - 1),
                            )
                            i += 1
                    kf = kpool.tile([128, b_per, 25], f32, tag=f"kf{cc}")
                    nc.scalar.activation(kf[:], psk[:], AF.Relu, bias=b1_sb[:, cc : cc + 1])
                    kf_sb.append(kf)
                    kfb = kpool.tile([128, b_per, 25], dt.bfloat16, tag=f"kfb{cc}")
                    nc.scalar.activation(kfb[:], psk[:], AF.Relu, bias=b1_sb[:, cc : cc + 1])
                    kfb_sb.append(kfb)

            diag_tiles = {}

            def build_diag(u):
                """diag(kf[:, b, t]) for all 25 taps: dg[c, t, j] = eye[c,j]*kf[c,b,t].

                One Pool broadcast tensor_tensor (~6.5us); Pool is otherwise
                idle so this never contends with evictions or DVE chains."""
                bb, cc = u // 2, u % 2
                dg = dpool.tile([128, 25, 128], dt.bfloat16, tag="diag")
                mask = eye_sb[:].unsqueeze(1).broadcast_to([128, 25, 128])
                data = kfb_sb[cc][:, bb].unsqueeze(2).broadcast_to([128, 25, 128])
                nc.gpsimd.tensor_tensor(dg[:], mask, data, ALU.mult)
                diag_tiles[u] = dg

            sf_tiles = {}
            feat_tiles = {}
            mq = {}  # unit -> list of (acc, win, kap, first)

            def drain_mq(n):
                # round-robin across units so the per-unit DMA-accumulate RAW
                # chains interleave: the Pool SEQ never sits blocked on one
                # chain's previous add while another chain could generate
                done = 0
                while done < n and mq:
                    for u in list(mq.keys()):
                        ops = mq[u]
                        acc, win, kap, first = ops.pop(0)
                        if not ops:
                            del mq[u]
                        if first:
                            nc.scalar.activation(acc[:], win, AF.Copy, scale=kap)
                        else:
                            P = ppool.tile([128, 25, 26], f32, tag="prod")
                            nc.scalar.activation(P[:], win, AF.Copy, scale=kap)
                            nc.gpsimd.dma_start(acc[:], P[:], accum_op=ALU.add)
                        done += 1
                        if done >= n:
                            break

            def emit_convs(b, mid_hook=None):
                sin = sin_tiles.pop(b)
                sf = []
                for cc in range(2):
                    if cc == 1 and mid_hook is not None:
                        mid_hook()
                    sft = sfpool.tile([128, 29, 30], dt.bfloat16, tag="sf")
                    for (r0, nr) in CS_ROWS:
                        pscs = ps_cs.tile([128, 15, 29], f32, tag="ps")
                        n_acc = len(TAPS3) * 2
                        i = 0
                        for ci in range(2):
                            for (dy, dx) in TAPS3:
                                t2c = (dy * 3 + dx) * 2 + cc
                                nc.tensor.matmul(
                                    pscs[:, :nr, :],
                                    ws_sb[ci][:, t2c, :],
                                    sin[ci][:, dy + r0 : dy + r0 + nr, dx : dx + 29],
                                    start=(i == 0),
                                    stop=(i == n_acc - 1),
                                )
                                i += 1
                        nc.scalar.activation(
                            sft[:, r0 : r0 + nr, 0:29],
                            pscs[:, :nr, :],
                            AF.Relu,
                            bias=b2_sb[:, cc : cc + 1],
                        )
                        drain_mq(7)
                    sf.append(sft)
                sf_tiles[b] = sf

            def emit_xcorr(b):
                sf = sf_tiles.pop(b)
                feat = [None, None]
                for cc in range(2):
                    u = b * 2 + cc
                    ft = fpool.tile([128, 25, 26], f32r, tag="feat")
                    if u in M_UNITS:
                        # ACT products + DMA-accumulate: queue the per-tap ops;
                        # they are drained a few at a time between evictions
                        for ti, (dy, dx) in enumerate(TAPS5):
                            kap = kf_sb[cc][:, b, dy * 5 + dx : dy * 5 + dx + 1]
                            win = sf[cc][:, dy : dy + 25, dx : dx + 26]
                            mq.setdefault(u, []).append((ft, win, kap, ti == 0))
                        feat[cc] = ft
                        continue
                    if u in PE_UNITS:
                        dg = diag_tiles.pop(u)
                        for (r0, nr) in H_ROWS:
                            psx = ps_h.tile([128, 13, 25], f32, tag="ph")
                            for ti, (dy, dx) in enumerate(TAPS5):
                                nc.tensor.matmul(
                                    psx[:, :nr, :],
                                    dg[:, ti, :],
                                    sf[cc][:, dy + r0 : dy + r0 + nr, dx : dx + 25],
                                    start=(ti == 0),
                                    stop=(ti == 24),
                                )
                            nc.scalar.activation(
                                ft[:, r0 : r0 + nr, 0:25], psx[:, :nr, :], AF.Copy
                            )
                    else:
                        dv = ft[:, :, 0:25]
                        for ti, (dy, dx) in enumerate(TAPS5):
                            kap = kf_sb[cc][:, b, dy * 5 + dx : dy * 5 + dx + 1]
                            win = sf[cc][:, dy : dy + 25, dx : dx + 25]
                            if ti == 0:
                                nc.vector.tensor_scalar(dv, win, kap, None, ALU.mult)
                            else:
                                nc.vector.scalar_tensor_tensor(
                                    dv, win, kap, dv, ALU.mult, ALU.add
                                )
                    feat[cc] = ft

                feat_tiles[b] = feat
                drain_mq(8)

            def emit_head(b):
                feat = feat_tiles.pop(b)
                # h1: 1x1 conv + bn3 + relu -> h1o [2][128, 25, 25]
                h1o = []
                for cc2 in range(2):
                    ht = hpool.tile([128, 25, 25], dt.bfloat16, tag="h1o")
                    for (r0, nr) in H_ROWS:
                        psh = ps_h.tile([128, 13, 26], f32, tag="ph")
                        for ci in range(2):
                            nc.tensor.matmul(
                                psh[:, :nr, :],
                                w1_sb[ci][:, cc2, :],
                                feat[ci][:, r0 : r0 + nr, :],
                                start=(ci == 0),
                                stop=(ci == 1),
                            )
                        nc.scalar.activation(
                            ht[:, r0 : r0 + nr, :],
                            psh[:, :nr, 0:25],
                            AF.Relu,
                            bias=b3_sb[:, cc2 : cc2 + 1],
                        )
                    h1o.append(ht)

                # h2: 1x1 conv (+bias) -> out [10, 25, 25]
                osb = opool.tile([10, 25, 25], f32, tag="osb")
                for (r0, nr) in H_ROWS:
                    psh2 = ps_h2.tile([10, 13, 25], f32, tag="ph2")
                    for ci in range(2):
                        nc.tensor.matmul(
                            psh2[:, :nr, :],
                            w2_sb[ci][:, :],
                            h1o[ci][:, r0 : r0 + nr, :],
                            start=(ci == 0),
                            stop=(ci == 1),
                        )
                    nc.scalar.activation(
                        osb[:, r0 : r0 + nr, :],
                        psh2[:, :nr, :],
                        AF.Identity,
                        bias=bh_sb[:, :],
                    )
                # y store on the ACT ring: osb was just evicted by ACT, so the
                # DMA dispatch never waits on a semaphore
                nc.scalar.dma_start(y_d[b], osb[:])

            # ---- software-pipelined sample loop (head lags by 2 samples).
            # The head is emitted BEFORE conv_search so its PSUM evictions sit
            # ahead of conv evictions in the in-order ACT queue (no
            # head-of-line blocking -> no PE stall on ps_h banks). ----
            # diag build schedule: 2 iterations ahead of use; when a sample
            # has two PE units, stagger the second to the next iteration so
            # only 2 diag tiles are ever live (dpool bufs=2)
            diag_sched = {}
            seen_samples = set()
            for u in PE_UNITS:
                it = u // 2 - 2
                if u // 2 in seen_samples:
                    it += 1
                seen_samples.add(u // 2)
                diag_sched.setdefault(max(it, 0), []).append(u)

            for b in range(b_per):
                if b + 2 < b_per:
                    emit_load(b + 2)
                emit_convs(b, mid_hook=emit_phase_k if b == 0 else None)
                for u in diag_sched.get(b, ()):
                    build_diag(u)
                if b == b_per - 1:
                    drain_mq(10**9)
                    # tail: drain two heads before the last xcorr so their
                    # PSUM ping-pong overlaps the xcorr matmuls
                    emit_head(b - 3)
                    emit_head(b - 2)
                    emit_xcorr(b)
                    # head[15]'s feats come from the PE itself (units 30/31),
                    # so run it first while DVE drains sample 14's units
                    emit_head(b)
                    emit_head(b - 1)
                else:
                    emit_xcorr(b)
                    if b >= 3:
                        emit_head(b - 3)

    nc.compile()
    return nc


def _get_nc(b_per=B_PER):
    key = b_per
    if key not in _NC_CACHE:
        _NC_CACHE[key] = _build_nc(b_per)
    return _NC_CACHE[key]


def _host_prep(inputs):
    """Fold BN into weights, transpose to lhsT layouts, slice per core."""
    import ml_dtypes

    bf16 = ml_dtypes.bfloat16
    f = np.float32
    kernel = np.ascontiguousarray(inputs["kernel"], dtype=f)
    search = np.ascontiguousarray(inputs["search"], dtype=f)

    def bn_fold(g, b_, m, v):
        scale = g / np.sqrt(v + EPS)
        shift = b_ - m * scale
        return scale.astype(f), shift.astype(f)

    s1, sh1 = bn_fold(inputs["g1"], inputs["b1"], inputs["m1"], inputs["v1"])
    s2, sh2 = bn_fold(inputs["g2"], inputs["b2"], inputs["m2"], inputs["v2"])
    s3, sh3 = bn_fold(inputs["g3"], inputs["b3"], inputs["m3"], inputs["v3"])

    def conv3_lhsT(w, scale):
        # w [co=256, ci=256, 3, 3] * scale[co] -> [cic, ci128, tap*2+coc, co128]
        wf = (w * scale[:, None, None, None]).astype(f)
        wf = wf.reshape(2, 128, 2, 128, 3, 3)  # [coc, co, cic, ci, dy, dx]
        wf = wf.transpose(2, 3, 4, 5, 0, 1)  # [cic, ci, dy, dx, coc, co]
        return np.ascontiguousarray(wf.reshape(2, 128, 18, 128))

    wk = conv3_lhsT(inputs["w_ck"], s1).astype(bf16)
    ws = conv3_lhsT(inputs["w_cs"], s2).astype(bf16)

    w1 = (inputs["w_h1"][:, :, 0, 0] * s3[:, None]).astype(f)  # [co 256, ci 256]
    w1 = w1.reshape(2, 128, 2, 128).transpose(2, 3, 0, 1)  # [cic, ci, coc, co]
    w1 = np.ascontiguousarray(w1)
    w2 = inputs["w_h2"][:, :, 0, 0].astype(f)  # [10, 256]
    w2 = np.ascontiguousarray(w2.reshape(10, 2, 128).transpose(1, 2, 0)).astype(bf16)  # [cic, ci, 10]

    weights = dict(
        wk=wk,
        ws=ws,
        w1=w1,
        w2=w2,
        eye=np.eye(128, dtype=bf16),
        b1s=np.ascontiguousarray(sh1.reshape(2, 128).T),
        b2s=np.ascontiguousarray(sh2.reshape(2, 128).T),
        b3s=np.ascontiguousarray(sh3.reshape(2, 128).T),
        bhs=np.ascontiguousarray(inputs["b_h2"].astype(f).reshape(10, 1)),
    )

    in_maps = []
    for c in range(N_CORES):
        sl = slice(c * B_PER, (c + 1) * B_PER)
        win = np.lib.stride_tricks.sliding_window_view(kernel[sl], (5, 5), axis=(2, 3))
        # win[b, c, dy, dx, y, x] = kernel[b, c, y+dy, x+dx]
        kin = win.reshape(B_PER, 2, 128, 9, 25).transpose(1, 2, 3, 0, 4)
        sp = np.zeros((B_PER, 2, 128, 31, 32), dtype=bf16)
        sp[..., :31] = search[sl].reshape(B_PER, 2, 128, 31, 31).astype(bf16)
        m = dict(weights)
        m["search"] = sp
        m["kin"] = np.ascontiguousarray(kin).astype(bf16)
        in_maps.append(m)
    return in_maps


def run(trace=False, **inputs):
    from concourse import bass_utils

    in_maps = _host_prep(inputs)
    nc = _get_nc()
    try:
        res = bass_utils.run_bass_kernel_spmd(
            nc, in_maps, core_ids=list(range(N_CORES)), trace=trace
        )
    except ModuleNotFoundError:
        # NTFF profiling hook unavailable in this container
        res = bass_utils.run_bass_kernel_spmd(
            nc, in_maps, core_ids=list(range(N_CORES)), trace=False
        )
    y = np.concatenate([res.results[c]["y"] for c in range(N_CORES)], axis=0)
    return y.reshape(B, 10, 25, 25), res


def kernel(**inputs):
    y, _ = run(trace=False, **inputs)
    return y
